# revision 5
# baseline (speedup 1.0000x reference)
"""AuxPEAttention Trainium2 kernel.

Full inputs -> full output. Sharding: 8 cores = batch(2) x head-groups(4).
Each core computes, for its batch b and its 4 heads (256 channels):
  QT = (Wq_g^T @ x^T) + (PEq gather + bq)^T        [256, T]  (bf16)
  KT = (Wk_g^T @ x_aug^T) + (PEk gather + bk)^T    [256, T]  (bf16)
  V  = x_aug_pad @ Wv_pad (ones row carries bv)    [T, 256]  (bf16)
  per head h, per q-strip of 512:
    S^T[k,q] = K_h Q_h^T  (row-packed head pairs, K=64)
    P = exp(S/8) * mask   (no max-subtraction; scores are O(1))
    OT_aug[65, q] = [V_h | 1]^T @ P  -> rows 0-63 = O^T, row 64 = denom
    OT_h = OT_aug[0:64] * exp(-ln denom)   (broadcast via gpsimd)
  partial[T, 1024] = OT^T @ Wo_g   (row-sharded Wo)
Host sums the 4 per-group partials per batch and adds bo.
"""
import numpy as np
import ml_dtypes

import concourse.tile as tile
from concourse import bacc, mybir
from concourse.bass_utils import run_bass_kernel_spmd

F32 = mybir.dt.float32
BF16 = mybir.dt.bfloat16
ACT = mybir.ActivationFunctionType

B, T, C, A = 2, 2048, 1024, 256
H, HD = 16, 64
G = 4              # head groups = tensor-parallel factor
HPG = H // G       # 4 heads per group
CG = HPG * HD      # 256 channels per group
CA = C + A         # 1280
CAP = 1408         # padded to 11*128 (row 1280 = ones for bv)
NQ = C // 128      # 8 contraction tiles for Q
NK = CA // 128     # 10 for K
NV = CAP // 128    # 11 for V
NKT = T // 128     # 16 key tiles
NQT = T // 512     # 4 query strips
SCALE = 1.0 / 8.0  # 1/sqrt(HD)

_CACHE = {}
LAST_RESULT = None
LAST_IN_MAPS = None


def _build():
    nc = bacc.Bacc(target_bir_lowering=False)
    xat_d = nc.dram_tensor("xat", [CAP, T], BF16, kind="ExternalInput")
    wq_d = nc.dram_tensor("wq", [C, CG], BF16, kind="ExternalInput")
    wk_d = nc.dram_tensor("wk", [CA, CG], BF16, kind="ExternalInput")
    wv_d = nc.dram_tensor("wv", [CAP, CG], BF16, kind="ExternalInput")
    wo_d = nc.dram_tensor("wo", [CG, C], BF16, kind="ExternalInput")
    aq_d = nc.dram_tensor("addq", [CG, T], F32, kind="ExternalInput")
    ak_d = nc.dram_tensor("addk", [CG, T], F32, kind="ExternalInput")
    mk_d = nc.dram_tensor("maskt", [NQT, 128, NKT, 512], BF16, kind="ExternalInput")
    out_d = nc.dram_tensor("out", [T, C], F32, kind="ExternalOutput")

    xat_r = xat_d.rearrange("(kt p) t -> p kt t", p=128)
    wq_r = wq_d.rearrange("(kt p) c -> p kt c", p=128)
    wk_r = wk_d.rearrange("(kt p) c -> p kt c", p=128)
    wv_r = wv_d.rearrange("(kt p) c -> p kt c", p=128)
    wo_r = wo_d.rearrange("(kt p) c -> p kt c", p=128)
    aq_r = aq_d.rearrange("(pt p) t -> p pt t", p=128)
    ak_r = ak_d.rearrange("(pt p) t -> p pt t", p=128)

    with tile.TileContext(nc) as tc:
        with tc.tile_pool(name="persist", bufs=1) as persist, \
             tc.tile_pool(name="ps_s", bufs=3, space="PSUM") as ps_s, \
             tc.tile_pool(name="ps_ot", bufs=2, space="PSUM") as ps_ot, \
             tc.tile_pool(name="small", bufs=4) as small, \
             tc.tile_pool(name="tmp64", bufs=2) as tmp64, \
             tc.tile_pool(name="maskp", bufs=2) as maskp, \
             tc.tile_pool(name="pstrip", bufs=3) as pstrip:
            qts = persist.tile([128, 2, T], BF16)
            kts = persist.tile([128, 2, T], BF16)
            vs = persist.tile([128, NKT, HPG, HD + 1], BF16)
            ots = persist.tile([128, 2, T], BF16)
            nc.vector.memset(vs[:, :, :, HD:HD + 1], 1.0)

            # ---------------- Phase 1: projections ----------------
            with tc.tile_pool(name="ph1", bufs=1) as ph1, \
                 tc.tile_pool(name="ph1s", bufs=3) as ph1s:
                xat = ph1.tile([128, NV, T], BF16)
                for kt in range(NV):
                    nc.sync.dma_start(out=xat[:, kt, :], in_=xat_r[:, kt, :])
                wq = ph1.tile([128, NQ, CG], BF16)
                wk = ph1.tile([128, NK, CG], BF16)
                wv = ph1.tile([128, NV, CG], BF16)
                nc.sync.dma_start(out=wq, in_=wq_r)
                nc.sync.dma_start(out=wk, in_=wk_r)
                nc.sync.dma_start(out=wv, in_=wv_r)

                # QT / KT: out = W^T @ xaT, evicted with +PE/bias add
                for dst, w, nk, add_r in (
                    (qts, wq, NQ, aq_r),
                    (kts, wk, NK, ak_r),
                ):
                    for pt in range(2):
                        for nt in range(NQT):
                            ps = ps_s.tile([128, 1024], F32, tag="s")
                            for kt in range(nk):
                                nc.tensor.matmul(
                                    ps[:, 0:512],
                                    w[:, kt, pt * 128:(pt + 1) * 128],
                                    xat[:, kt, nt * 512:(nt + 1) * 512],
                                    start=(kt == 0), stop=(kt == nk - 1),
                                )
                            add_t = ph1s.tile([128, 512], F32, tag="add")
                            nc.sync.dma_start(
                                out=add_t,
                                in_=add_r[:, pt, nt * 512:(nt + 1) * 512])
                            nc.vector.tensor_add(
                                dst[:, pt, nt * 512:(nt + 1) * 512],
                                ps[:, 0:512], add_t)

                # V: out[T,256] tiles, partition = tokens
                for mt in range(NKT):
                    ps = ps_s.tile([128, 1024], F32, tag="s")
                    for kt in range(NV):
                        nc.tensor.matmul(
                            ps[:, 0:256],
                            xat[:, kt, mt * 128:(mt + 1) * 128],
                            wv[:, kt, :],
                            start=(kt == 0), stop=(kt == NV - 1),
                        )
                    nc.vector.tensor_copy(
                        vs[:, mt, :, 0:HD],
                        ps[:, 0:256].rearrange("p (h d) -> p h d", h=HPG))

            # ---------------- Phase 2: attention ----------------
            for qt in range(NQT):
                mstrip = maskp.tile([128, NKT, 512], BF16, tag="m")
                nc.sync.dma_start(out=mstrip, in_=mk_d.ap()[qt])
                for hp in range(2):
                    ptiles = [
                        pstrip.tile([128, NKT, 512], BF16, tag="p",
                                    name=f"p_{qt}_{hp}_{j}")
                        for j in range(2)
                    ]
                    # scores + exp, head pair row-packed (K=64 at rows 0/64)
                    for g2 in range(NKT // 2):
                        pss = [ps_s.tile([128, 1024], F32, tag="s",
                                         name=f"ss_{qt}_{hp}_{g2}_{i}")
                               for i in range(2)]
                        for j in range(2):
                            for h01 in range(2):
                                kt = 2 * g2 + j
                                r0 = 64 * h01
                                nc.tensor.matmul(
                                    pss[h01][:, j * 512:(j + 1) * 512],
                                    kts[r0:r0 + 64, hp, kt * 128:(kt + 1) * 128],
                                    qts[r0:r0 + 64, hp, qt * 512:(qt + 1) * 512],
                                    start=True, stop=True,
                                )
                        for h01 in range(2):
                            nc.scalar.activation(
                                out=ptiles[h01][:, 2 * g2:2 * g2 + 2, :],
                                in_=pss[h01][:, :],
                                func=ACT.Exp, scale=SCALE)
                    # mask multiply (quarter granularity for pipelining)
                    for h01 in range(2):
                        for q4 in range(4):
                            sl = slice(4 * q4, 4 * q4 + 4)
                            nc.vector.tensor_mul(
                                ptiles[h01][:, sl, :],
                                ptiles[h01][:, sl, :],
                                mstrip[:, sl, :])
                    # PV + normalize
                    for h01 in range(2):
                        h = 2 * hp + h01
                        otp = ps_ot.tile([HD + 1, 512], F32, tag="ot")
                        for kt in range(NKT):
                            nc.tensor.matmul(
                                otp, vs[:, kt, h, :], ptiles[h01][:, kt, :],
                                start=(kt == 0), stop=(kt == NKT - 1))
                        lnd = small.tile([1, 512], F32, tag="ln")
                        rcp = small.tile([1, 512], F32, tag="rc")
                        bc = small.tile([64, 512], F32, tag="bc")
                        nc.scalar.activation(out=lnd, in_=otp[HD:HD + 1, :],
                                             func=ACT.Ln)
                        nc.scalar.activation(out=rcp, in_=lnd,
                                             func=ACT.Exp, scale=-1.0)
                        nc.gpsimd.partition_broadcast(bc, rcp)
                        qsl = slice(qt * 512, (qt + 1) * 512)
                        if h01 == 0:
                            nc.vector.tensor_mul(
                                ots[0:64, hp, qsl], otp[0:HD, :], bc)
                        else:
                            tmp = tmp64.tile([64, 512], BF16, tag="t")
                            nc.vector.tensor_mul(tmp, otp[0:HD, :], bc)
                            nc.sync.dma_start(out=ots[64:128, hp, qsl], in_=tmp)

            # ---------------- Phase 3: output projection ----------------
            with tc.tile_pool(name="ph3", bufs=1) as ph3, \
                 tc.tile_pool(name="ph3o", bufs=3) as ph3o:
                wo = ph3.tile([128, 2, C], BF16)
                nc.sync.dma_start(out=wo, in_=wo_r)
                for mt in range(NKT):
                    ps = ps_s.tile([128, 1024], F32, tag="s")
                    for nt in range(2):
                        for pt in range(2):
                            nc.tensor.matmul(
                                ps[:, nt * 512:(nt + 1) * 512],
                                ots[:, pt, mt * 128:(mt + 1) * 128],
                                wo[:, pt, nt * 512:(nt + 1) * 512],
                                start=(pt == 0), stop=(pt == 1))
                    osb = ph3o.tile([128, C], F32, tag="o")
                    nc.vector.tensor_copy(osb, ps)
                    nc.sync.dma_start(
                        out=out_d.ap()[mt * 128:(mt + 1) * 128, :], in_=osb)

    nc.finalize()
    return nc


def kernel(**inputs):
    global LAST_RESULT, LAST_IN_MAPS
    x = np.asarray(inputs["x"], dtype=np.float32)
    aux_x = np.asarray(inputs["aux_x"], dtype=np.float32)
    Wq = np.asarray(inputs["Wq"], dtype=np.float32)
    bq = np.asarray(inputs["bq"], dtype=np.float32)
    Wk = np.asarray(inputs["Wk"], dtype=np.float32)
    bk = np.asarray(inputs["bk"], dtype=np.float32)
    Wv = np.asarray(inputs["Wv"], dtype=np.float32)
    bv = np.asarray(inputs["bv"], dtype=np.float32)
    Wo = np.asarray(inputs["Wo"], dtype=np.float32)
    bo = np.asarray(inputs["bo"], dtype=np.float32)
    PEq = np.asarray(inputs["PEq"], dtype=np.float32)
    PEk = np.asarray(inputs["PEk"], dtype=np.float32)
    mask = np.asarray(inputs["attn_mask"])
    fi = np.asarray(inputs["frame_indices"]).astype(np.int64)

    bf = ml_dtypes.bfloat16
    # per-batch shared tensors
    xat_b, maskt_b, peq_b, pek_b = [], [], [], []
    for b in range(B):
        xat = np.zeros((CAP, T), dtype=np.float32)
        xat[:C] = x[b].T
        xat[C:CA] = aux_x[b].T
        xat[CA] = 1.0
        xat_b.append(xat.astype(bf))
        mt = mask[b].T.astype(np.float32)  # [k, q]
        maskt_b.append(np.ascontiguousarray(
            mt.reshape(NKT, 128, NQT, 512).transpose(2, 1, 0, 3)).astype(bf))
        peq_b.append(PEq[fi[b]])  # [T, C]
        pek_b.append(PEk[fi[b]])

    in_maps = []
    for c in range(8):
        b, g = divmod(c, G)
        cols = slice(g * CG, (g + 1) * CG)
        wv_pad = np.zeros((CAP, CG), dtype=np.float32)
        wv_pad[:CA] = Wv[:, cols]
        wv_pad[CA] = bv[cols]
        in_maps.append({
            "xat": xat_b[b],
            "wq": np.ascontiguousarray(Wq[:, cols]).astype(bf),
            "wk": np.ascontiguousarray(Wk[:, cols]).astype(bf),
            "wv": wv_pad.astype(bf),
            "wo": np.ascontiguousarray(Wo[cols, :]).astype(bf),
            "addq": np.ascontiguousarray((peq_b[b][:, cols] + bq[cols]).T),
            "addk": np.ascontiguousarray((pek_b[b][:, cols] + bk[cols]).T),
            "maskt": maskt_b[b],
        })

    global LAST_IN_MAPS
    LAST_IN_MAPS = in_maps
    if "nc" not in _CACHE:
        _CACHE["nc"] = _build()
    res = run_bass_kernel_spmd(_CACHE["nc"], in_maps, core_ids=list(range(8)))
    LAST_RESULT = res

    out = np.zeros((B, T, C), dtype=np.float32)
    for c in range(8):
        b = c // G
        out[b] += res.results[c]["out"]
    out += bo
    return out
